# revision 10
# baseline (speedup 1.0000x reference)
"""AdaAggLayer Trainium2 kernel.

Data-parallel over batch: 8 NeuronCores x 4 samples each.

The align transform (w_aligned[e] = align[e] @ w[e]) is a weight-only
reparameterization, so it is folded on the host (float32, exact) and the
aligned weights are DMA'd in directly, transposed to the [i, o] layout the
conv needs. Per core:
  - attention (global avg pool -> 1x1 -> relu -> 1x1 -> sigmoid) on PE/ACT/DVE;
    sample 0's pooling is chunked so it starts while its x is still landing
  - per-sample weight aggregation sum_e att[b,e]*walT[e] on DVE (bf16,
    contiguous chunks so tensor_scalar hits 4x mode and tensor_tensor 2x)
  - per-sample 3x3 conv as 9 shifted matmuls accumulating in PSUM (bf16),
    7 row-blocks interleaved across 7 PSUM banks
  - bias epilogue fused into the PSUM->SBUF copy on ACT, output stored bf16
No collectives: inputs are sharded/replicated host-side, outputs concatenated.
"""

import contextlib
import importlib.util
import sys
import types

sys.path.insert(0, "/opt/trn_rl_repo")

import numpy as np
import ml_dtypes

import concourse.bass as bass
import concourse.mybir as mybir
import concourse.tile as tile
from concourse import bacc
from concourse.bass_utils import run_bass_kernel_spmd

N_CORES = 8
B, I, O, E, HID = 32, 256, 256, 5, 65
H = W = 56
HP = H + 2  # zero-padded spatial
BL = B // N_CORES  # samples per core
KK = 9  # 3x3 taps
NG = 3  # kk groups of 3
NBLK = 7  # row blocks of 8 output rows
RB = 8  # rows per block
BF16 = mybir.dt.bfloat16
F32 = mybir.dt.float32

# row chunks for sample 0's pooled-sum pipeline (reduce each as it lands)
X0_CHUNKS = [(0, 15), (15, 30), (30, 45), (45, 58)]

_NC_CACHE = None


def _install_ntff_hook():
    """Register the axon NTFF profiling hook (the image's antenv lacks it)."""
    if "antenv.axon_hooks" in sys.modules:
        return
    try:
        spec = importlib.util.spec_from_file_location(
            "trn_boot", "/root/.axon_site/trn_agent_boot/trn_boot.py"
        )
        tb = importlib.util.module_from_spec(spec)
        spec.loader.exec_module(tb)
        hook = tb._ntff_profile_via_ctypes("/opt/axon/libaxon_pjrt.so")
    except Exception:
        hook = None
    mod = types.ModuleType("antenv.axon_hooks")
    mod.get_axon_ntff_profile_hook = lambda: hook
    sys.modules["antenv.axon_hooks"] = mod


def _emit(nc, tc, ctx):
    x_d = nc.dram_tensor("x", [BL, I, HP, HP], BF16, kind="ExternalInput")
    # aligned+transposed weights, chunk-ordered: [kkg, ot, p=i%128, e, kq, ih, o128]
    w_d = nc.dram_tensor("walT", [NG, 2, 128, E, 3, 2, 128], BF16, kind="ExternalInput")
    w1_d = nc.dram_tensor("w1T", [I, HID], F32, kind="ExternalInput")
    # attn_w2.T with attn_b2 appended as a trailing row; paired with a
    # constant-1 row in h so the second 1x1 conv's bias rides the matmul.
    w2_d = nc.dram_tensor("w2Ta", [HID + 1, E], F32, kind="ExternalInput")
    bias_d = nc.dram_tensor("bias", [E, O], F32, kind="ExternalInput")
    out_d = nc.dram_tensor("out", [BL, O, H, W], BF16, kind="ExternalOutput")

    const = ctx.enter_context(tc.tile_pool(name="const", bufs=1))
    xpool = ctx.enter_context(tc.tile_pool(name="x", bufs=1))
    aggp = ctx.enter_context(tc.tile_pool(name="agg", bufs=1))
    tmpp = ctx.enter_context(tc.tile_pool(name="tmp", bufs=2))
    stagep = ctx.enter_context(tc.tile_pool(name="stage", bufs=4))
    s_psum = ctx.enter_context(tc.tile_pool(name="sps", bufs=1, space="PSUM"))
    c_psum = ctx.enter_context(tc.tile_pool(name="cps", bufs=7, space="PSUM"))

    # ---- SBUF constants ----
    # walT[p, kkg, ot, e, kq, ih, o]: each (kkg, ot, e) run is contiguous so
    # the aggregation DVE ops stream stride-1 (4x/2x perf modes).
    walT = const.tile([128, NG, 2, E, 3, 2, 128], BF16)
    warm_w = const.tile([128, 128], BF16)  # PE warm-up operands
    warm_x = const.tile([128, 448], BF16)
    pscr = const.tile([128, 2, HP, HP], BF16)  # pooling accum-trick scratch
    w1_sb = const.tile([128, 2, HID], F32)  # part = i % 128
    w2_sb = const.tile([HID + 1, E], F32)
    bias_sb = const.tile([E, O], F32)
    ones_sb = const.tile([1, 128], F32)
    pp_sb = const.tile([128, 2, len(X0_CHUNKS)], F32)  # b0 partial pooled sums
    pooledT = const.tile([128, 2, BL], F32)  # part = i % 128
    h_sb = const.tile([HID + 1, BL], F32)  # row HID is constant 1.0
    att_sb = const.tile([E, BL], F32)
    att_row = const.tile([1, BL * E], F32)
    att_bc = const.tile([128, BL, E], F32)
    aggb_sb = const.tile([128, 2, BL], F32)  # part = o % 128

    x_sb = {}

    def dma_x(b, chunks=None):
        for ih in range(2):
            t = xpool.tile([128, HP, HP], BF16, tag=f"x{b}_{ih}", name=f"x{b}_{ih}")
            if chunks is None:
                nc.sync.dma_start(out=t[:, :, :], in_=x_d[b, ih * 128 : (ih + 1) * 128, :, :])
            else:
                for r0, r1 in chunks:
                    nc.sync.dma_start(
                        out=t[:, r0:r1, :],
                        in_=x_d[b, ih * 128 : (ih + 1) * 128, r0:r1, :],
                    )
            x_sb[(b, ih)] = t

    def attention_tail(b0, nb, hp):
        # shared epilogue once hp[:, :nb] holds the pre-relu hidden acts
        nc.scalar.activation(
            h_sb[:HID, b0 : b0 + nb], hp[:, :nb], mybir.ActivationFunctionType.Relu
        )
        ap = s_psum.tile([E, BL], F32, tag="sps", name="ap")
        nc.tensor.matmul(ap[:, :nb], lhsT=w2_sb[:, :], rhs=h_sb[:, b0 : b0 + nb])
        nc.scalar.activation(
            att_sb[:, b0 : b0 + nb], ap[:, :nb], mybir.ActivationFunctionType.Sigmoid
        )
        # att row per sample on partition 0 (M=1 matmul), then one broadcast
        # matmul to all 128 partitions -- no SBUF-to-SBUF DMA involved.
        rp = s_psum.tile([1, BL * E], F32, tag="sps", name="rp")
        for j in range(nb):
            b = b0 + j
            nc.tensor.matmul(
                rp[0:1, j * E : (j + 1) * E],
                lhsT=h_sb[:, b : b + 1],
                rhs=w2_sb[:, :],
            )
        nc.scalar.activation(
            att_row[0:1, b0 * E : (b0 + nb) * E],
            rp[0:1, : nb * E],
            mybir.ActivationFunctionType.Sigmoid,
        )
        bp = s_psum.tile([128, BL * E], F32, tag="sps", name="bp")
        nc.tensor.matmul(
            bp[:, : nb * E],
            lhsT=ones_sb[0:1, :],
            rhs=att_row[0:1, b0 * E : (b0 + nb) * E],
        )
        nc.vector.tensor_copy(out=att_bc[:, b0 : b0 + nb, :], in_=bp[:, : nb * E])
        # aggregated bias agg_b[o, b] = sum_e att[e,b] * bias[e, o]
        for ot in range(2):
            gp = s_psum.tile([128, BL], F32, tag="sps", name="gp")
            nc.tensor.matmul(
                gp[:, :nb],
                lhsT=bias_sb[:, ot * 128 : (ot + 1) * 128],
                rhs=att_sb[:, b0 : b0 + nb],
            )
            nc.vector.tensor_copy(out=aggb_sb[:, ot, b0 : b0 + nb], in_=gp[:, :nb])

    def pool_half(b, ih):
        # pooled sum via tensor_scalar's accum_out: the copy runs in 4x bf16
        # mode (vs 1x for tensor_reduce), accum_out is the per-partition sum.
        nc.vector.tensor_scalar(
            out=pscr[:, ih, :, :],
            in0=x_sb[(b, ih)][:, :, :],
            scalar1=1.0,
            scalar2=None,
            op0=mybir.AluOpType.mult,
            op1=mybir.AluOpType.add,
            accum_out=pooledT[:, ih, b : b + 1],
        )

    def attention0():
        for ih in range(2):
            pool_half(0, ih)
        hp = s_psum.tile([HID, BL], F32, tag="sps", name="hp")
        for ih in range(2):
            nc.tensor.matmul(
                hp[:, 0:1],
                lhsT=w1_sb[:, ih, :],
                rhs=pooledT[:, ih, 0:1],
                start=(ih == 0),
                stop=(ih == 1),
            )
        attention_tail(0, 1, hp)

    def attention123():
        for b in range(1, BL):
            for ih in range(2):
                pool_half(b, ih)
        hp = s_psum.tile([HID, BL], F32, tag="sps", name="hp")
        for ih in range(2):
            nc.tensor.matmul(
                hp[:, :3],
                lhsT=w1_sb[:, ih, :],
                rhs=pooledT[:, ih, 1:4],
                start=(ih == 0),
                stop=(ih == 1),
            )
        attention_tail(1, 3, hp)

    # per-sample aggregation chunks: scale+add tree on DVE --
    # tensor_scalar (4x bf16) + tensor_tensor (2x bf16) beats the 1x-mode
    # fused scalar_tensor_tensor chain. Chunked by (3 kk, ot-half) so each
    # conv only waits for its own chunks and pipelines with them.
    aggs_all = {}

    def agg_chunk(b, g, ot, split=False):
        # split=True: produce the chunk one kq at a time so the first conv
        # matmul can start after a third of the DVE work (head latency).
        agg = aggp.tile(
            [128, 3, 2, 128], BF16, tag=f"agg{b}_{g}_{ot}", name=f"agg{b}_{g}_{ot}"
        )
        kqs = [(kq, kq + 1) for kq in range(3)] if split else [(0, 3)]
        for q0, q1 in kqs:
            nc.vector.tensor_scalar_mul(
                agg[:, q0:q1, :, :], walT[:, g, ot, 0, q0:q1, :, :], att_bc[:, b, 0:1]
            )
            for e in range(1, E):
                tmp = tmpp.tile([128, 3, 2, 128], BF16, tag="tmp", name="tmp")
                nc.vector.tensor_scalar_mul(
                    tmp[:, q0:q1, :, :],
                    walT[:, g, ot, e, q0:q1, :, :],
                    att_bc[:, b, e : e + 1],
                )
                nc.vector.tensor_add(
                    out=agg[:, q0:q1, :, :],
                    in0=agg[:, q0:q1, :, :],
                    in1=tmp[:, q0:q1, :, :],
                )
        for kq in range(3):
            aggs_all[(b, g * 3 + kq, ot)] = (agg, kq)

    # ---- DMA schedule: x0 first (attention head), then consts, then the
    # aligned weights in aggregation-chunk order, then the rest of x.
    dma_x(0, chunks=X0_CHUNKS)
    for ih in range(2):
        nc.sync.dma_start(out=w1_sb[:, ih, :], in_=w1_d[ih * 128 : (ih + 1) * 128, :])
    nc.sync.dma_start(out=w2_sb[:, :], in_=w2_d[:, :])
    nc.sync.dma_start(out=bias_sb[:, :], in_=bias_d[:, :])
    nc.gpsimd.memset(warm_w[:, :], 0.0)
    nc.gpsimd.memset(warm_x[:, :], 0.0)
    nc.vector.memset(ones_sb[:, :], 1.0)
    # partition starts must be 32-aligned: memset rows 64-65, relu later
    # overwrites row 64 with real h values; row 65 stays the constant 1.0.
    nc.vector.memset(h_sb[HID - 1 : HID + 1, :], 1.0)
    for ot in range(2):
        for g in range(NG):
            for e in range(E):
                nc.sync.dma_start(
                    out=walT[:, g, ot, e, :, :, :], in_=w_d[g, ot, :, e, :, :, :]
                )
    for b in range(1, BL):
        dma_x(b)

    # preload the sigmoid ACT table off the critical path (a table switch
    # costs 1.3us; done here it hides under the input DMA).
    tscr = const.tile([1, 4], F32, name="tscr")
    nc.scalar.activation(
        tscr[0:1, :], ones_sb[0:1, 0:4], mybir.ActivationFunctionType.Sigmoid
    )

    def warm(n):
        # dummy matmuls keep the PE p-state ramped while the attention head
        # resolves, so the conv stream starts at full clock.
        for _ in range(n):
            wp = s_psum.tile([128, 448], F32, tag="sps", name="wp")
            nc.tensor.matmul(wp[:, :], lhsT=warm_w[:, :], rhs=warm_x[:, :])

    warm(34)
    attention0()
    warm(8)
    agg_chunk(0, 0, 0, split=True)
    for g in range(1, NG):
        agg_chunk(0, g, 0)
    for g in range(NG):
        agg_chunk(0, g, 1)

    # ---- per-sample: aggregate weights (DVE, kk-chunked) then conv (PE) ----
    for b in range(BL):
        for ot in range(2):
            for g in range(NG):
                if (b, g * 3, ot) not in aggs_all:
                    agg_chunk(b, g, ot)

        for ot in range(2):
            # all 7 row blocks interleave across 7 PSUM banks: one weight
            # stream per (kk, ih), PE accumulates to a different bank between
            # reloads so LDWEIGHTS is amortized 7x.
            cps = {
                blk: c_psum.tile([128, RB, W], F32, tag="cps", name=f"cp{b}_{ot}_{blk}")
                for blk in range(NBLK)
            }
            for kk in range(KK):
                agg_t, kq = aggs_all[(b, kk, ot)]
                di, dj = kk // 3 - 1, kk % 3 - 1
                for ih in range(2):
                    for blk in range(NBLK):
                        r0 = blk * RB
                        nc.tensor.matmul(
                            cps[blk][:, :, :],
                            lhsT=agg_t[:, kq, ih, :],
                            rhs=x_sb[(b, ih)][
                                :,
                                r0 + di + 1 : r0 + di + 1 + RB,
                                dj + 1 : dj + 1 + W,
                            ],
                            start=(kk == 0 and ih == 0),
                            stop=(kk == KK - 1 and ih == 1),
                        )
            for blk in range(NBLK):
                r0 = blk * RB
                st = stagep.tile([128, RB, W], BF16, tag="stage", name="st")
                nc.scalar.activation(
                    st[:, :, :],
                    cps[blk][:, :, :],
                    mybir.ActivationFunctionType.Identity,
                    bias=aggb_sb[:, ot, b : b + 1],
                )
                nc.sync.dma_start(
                    out=out_d[b, ot * 128 : (ot + 1) * 128, r0 : r0 + RB, :],
                    in_=st[:, :, :],
                )
            # remaining samples' attention rides inside conv0's stream:
            # their x has landed by then, and it finishes long before
            # conv0 ends so agg(b+1..) (DVE) overlaps the conv tails.
            if b == 0 and ot == 0:
                attention123()


def _build():
    nc = bacc.Bacc("TRN2", target_bir_lowering=False, debug=False, num_devices=N_CORES)
    with contextlib.ExitStack() as ctx:
        tc = ctx.enter_context(tile.TileContext(nc))
        _emit(nc, tc, ctx)
    nc.compile()
    return nc


def _get_nc():
    global _NC_CACHE
    if _NC_CACHE is None:
        _NC_CACHE = _build()
    return _NC_CACHE


def _run(trace=False, **inputs):
    x = np.asarray(inputs["x"], np.float32)
    weight = np.asarray(inputs["weight"], np.float32)
    bias = np.asarray(inputs["bias"], np.float32)
    align = np.asarray(inputs["align"], np.float32)
    w1 = np.asarray(inputs["attn_w1"], np.float32)
    w2 = np.asarray(inputs["attn_w2"], np.float32)
    b2 = np.asarray(inputs["attn_b2"], np.float32)

    xp = np.zeros((B, I, HP, HP), dtype=ml_dtypes.bfloat16)
    xp[:, :, 1 : 1 + H, 1 : 1 + W] = x

    # fold align into the expert weights (weight-only reparameterization),
    # then lay out transposed + chunk-ordered for the DMA schedule:
    # walT[kkg, ot, p, e, kq, ih, o] = w_al[e, ot*128+o, ih*128+p, kkg*3+kq]
    w_al = np.einsum("eno,eok->enk", align, weight.reshape(E, O, I * KK)).reshape(
        E, 2, 128, 2, 128, 3, 3
    )  # [e, ot, o, ih, p, kkg, kq]
    walT = np.ascontiguousarray(w_al.transpose(5, 1, 4, 0, 6, 3, 2)).astype(
        ml_dtypes.bfloat16
    )

    w1T = np.ascontiguousarray((w1 / float(H * W)).T)
    w2Ta = np.ascontiguousarray(
        np.concatenate([w2.T, b2.reshape(1, E)], axis=0)
    ).astype(np.float32)

    nc = _get_nc()
    in_maps = []
    for c in range(N_CORES):
        in_maps.append(
            {
                "x": xp[c * BL : (c + 1) * BL],
                "walT": walT,
                "w1T": w1T,
                "w2Ta": w2Ta,
                "bias": bias,
            }
        )
    if trace:
        _install_ntff_hook()
    res = run_bass_kernel_spmd(
        nc, in_maps, core_ids=list(range(N_CORES)), trace=trace
    )
    out = np.concatenate([res.results[c]["out"] for c in range(N_CORES)], axis=0)
    return out.astype(np.float32), res


def kernel(**inputs):
    out, _ = _run(trace=False, **inputs)
    return out


def kernel_profiled(**inputs):
    out, res = _run(trace=True, **inputs)
    return out, res


# revision 18
# speedup vs baseline: 1.0624x; 1.0624x over previous
"""AdaAggLayer Trainium2 kernel.

Data-parallel over batch: 8 NeuronCores x 4 samples each.

The align transform (w_aligned[e] = align[e] @ w[e]) is a weight-only
reparameterization, so it is folded on the host (float32, exact) and the
aligned weights are DMA'd in directly, transposed to the [i, o] layout the
conv needs. Per core:
  - attention (global avg pool -> 1x1 -> relu -> 1x1 -> sigmoid) on PE/ACT/DVE;
    sample 0's pooling is chunked so it starts while its x is still landing
  - per-sample weight aggregation sum_e att[b,e]*walT[e] on DVE (bf16,
    contiguous chunks so tensor_scalar hits 4x mode and tensor_tensor 2x)
  - per-sample 3x3 conv as 9 shifted matmuls accumulating in PSUM (bf16),
    7 row-blocks interleaved across 7 PSUM banks
  - bias epilogue fused into the PSUM->SBUF copy on ACT, output stored bf16
No collectives: inputs are sharded/replicated host-side, outputs concatenated.
"""

import contextlib
import importlib.util
import sys
import types

sys.path.insert(0, "/opt/trn_rl_repo")

import numpy as np
import ml_dtypes

import concourse.bass as bass
import concourse.mybir as mybir
import concourse.tile as tile
from concourse import bacc
from concourse.bass_utils import run_bass_kernel_spmd

N_CORES = 8
B, I, O, E, HID = 32, 256, 256, 5, 65
H = W = 56
HP = H + 2  # zero-padded spatial
BL = B // N_CORES  # samples per core
KK = 9  # 3x3 taps
NG = 3  # kk groups of 3
NBLK = 7  # row blocks of 8 output rows
RB = 8  # rows per block
BF16 = mybir.dt.bfloat16
F32 = mybir.dt.float32

# row chunks for sample 0's pooled-sum pipeline (reduce each as it lands)
X0_CHUNKS = [(0, 15), (15, 30), (30, 45), (45, 58)]

_NC_CACHE = None


def _install_ntff_hook():
    """Register the axon NTFF profiling hook (the image's antenv lacks it)."""
    if "antenv.axon_hooks" in sys.modules:
        return
    try:
        spec = importlib.util.spec_from_file_location(
            "trn_boot", "/root/.axon_site/trn_agent_boot/trn_boot.py"
        )
        tb = importlib.util.module_from_spec(spec)
        spec.loader.exec_module(tb)
        hook = tb._ntff_profile_via_ctypes("/opt/axon/libaxon_pjrt.so")
    except Exception:
        hook = None
    mod = types.ModuleType("antenv.axon_hooks")
    mod.get_axon_ntff_profile_hook = lambda: hook
    sys.modules["antenv.axon_hooks"] = mod


def _emit(nc, tc, ctx):
    x_d = nc.dram_tensor("x", [BL, I, HP, HP], BF16, kind="ExternalInput")
    # aligned+transposed weights, chunk-ordered: [kkg, ot, p=i%128, e, kq, ih, o128]
    w_d = nc.dram_tensor("walT", [NG, 2, 128, E, 3, 2, 128], BF16, kind="ExternalInput")
    w1_d = nc.dram_tensor("w1T", [I, HID], F32, kind="ExternalInput")
    # attn_w2.T with attn_b2 appended as a trailing row; paired with a
    # constant-1 row in h so the second 1x1 conv's bias rides the matmul.
    w2_d = nc.dram_tensor("w2Ta", [HID + 1, E], F32, kind="ExternalInput")
    bias_d = nc.dram_tensor("bias", [E, O], F32, kind="ExternalInput")
    out_d = nc.dram_tensor("out", [BL, O, H, W], BF16, kind="ExternalOutput")

    const = ctx.enter_context(tc.tile_pool(name="const", bufs=1))
    xpool = ctx.enter_context(tc.tile_pool(name="x", bufs=1))
    aggp = ctx.enter_context(tc.tile_pool(name="agg", bufs=1))
    tmpp = ctx.enter_context(tc.tile_pool(name="tmp", bufs=2))
    stagep = ctx.enter_context(tc.tile_pool(name="stage", bufs=4))
    s_psum = ctx.enter_context(tc.tile_pool(name="sps", bufs=1, space="PSUM"))
    c_psum = ctx.enter_context(tc.tile_pool(name="cps", bufs=7, space="PSUM"))

    # ---- SBUF constants ----
    # walT[p, kkg, ot, e, kq, ih, o]: each (kkg, ot, e) run is contiguous so
    # the aggregation DVE ops stream stride-1 (4x/2x perf modes).
    walT = const.tile([128, NG, 2, E, 3, 2, 128], BF16)
    pscr = const.tile([128, 2, 29, HP], BF16)  # pooling tree-reduce scratch
    w1_sb = const.tile([128, 2, HID], F32)  # part = i % 128
    w2_sb = const.tile([HID + 1, E], F32)
    bias_sb = const.tile([E, O], F32)
    ones_sb = const.tile([1, 128], F32)
    pooledT = const.tile([128, 2, BL], F32)  # part = i % 128
    h_sb = const.tile([HID + 1, BL], F32)  # row HID is constant 1.0
    att_sb = const.tile([E, BL], F32)
    att_row = const.tile([1, BL * E], F32)
    att_bc = const.tile([128, BL, E], F32)
    aggb_sb = const.tile([128, 2, BL], F32)  # part = o % 128

    x_sb = {}

    def dma_x(b, chunks=None):
        for ih in range(2):
            t = xpool.tile([128, HP, HP], BF16, tag=f"x{b}_{ih}", name=f"x{b}_{ih}")
            if chunks is None:
                nc.sync.dma_start(out=t[:, :, :], in_=x_d[b, ih * 128 : (ih + 1) * 128, :, :])
            else:
                for r0, r1 in chunks:
                    nc.sync.dma_start(
                        out=t[:, r0:r1, :],
                        in_=x_d[b, ih * 128 : (ih + 1) * 128, r0:r1, :],
                    )
            x_sb[(b, ih)] = t

    def attention_tail(b0, nb, hp):
        # shared epilogue once hp[:, :nb] holds the pre-relu hidden acts
        nc.scalar.activation(
            h_sb[:HID, b0 : b0 + nb], hp[:, :nb], mybir.ActivationFunctionType.Relu
        )
        ap = s_psum.tile([E, BL], F32, tag="sps", name="ap")
        nc.tensor.matmul(ap[:, :nb], lhsT=w2_sb[:, :], rhs=h_sb[:, b0 : b0 + nb])
        nc.scalar.activation(
            att_sb[:, b0 : b0 + nb], ap[:, :nb], mybir.ActivationFunctionType.Sigmoid
        )
        # att row per sample on partition 0 (M=1 matmul), then one broadcast
        # matmul to all 128 partitions -- no SBUF-to-SBUF DMA involved.
        rp = s_psum.tile([1, BL * E], F32, tag="sps", name="rp")
        for j in range(nb):
            b = b0 + j
            nc.tensor.matmul(
                rp[0:1, j * E : (j + 1) * E],
                lhsT=h_sb[:, b : b + 1],
                rhs=w2_sb[:, :],
            )
        nc.scalar.activation(
            att_row[0:1, b0 * E : (b0 + nb) * E],
            rp[0:1, : nb * E],
            mybir.ActivationFunctionType.Sigmoid,
        )
        bp = s_psum.tile([128, BL * E], F32, tag="sps", name="bp")
        nc.tensor.matmul(
            bp[:, : nb * E],
            lhsT=ones_sb[0:1, :],
            rhs=att_row[0:1, b0 * E : (b0 + nb) * E],
        )
        nc.vector.tensor_copy(out=att_bc[:, b0 : b0 + nb, :], in_=bp[:, : nb * E])
        # aggregated bias agg_b[o, b] = sum_e att[e,b] * bias[e, o]
        for ot in range(2):
            gp = s_psum.tile([128, BL], F32, tag="sps", name="gp")
            nc.tensor.matmul(
                gp[:, :nb],
                lhsT=bias_sb[:, ot * 128 : (ot + 1) * 128],
                rhs=att_sb[:, b0 : b0 + nb],
            )
            nc.vector.tensor_copy(out=aggb_sb[:, ot, b0 : b0 + nb], in_=gp[:, :nb])

    def pool_half(b, ih):
        # pooled sum as a pairwise row-fold tree: the adds run in 2x bf16 DVE
        # mode, beating the 1x tensor_reduce (~2.5us vs 3.6us per half).
        s = pscr[:, ih]
        xh = x_sb[(b, ih)]
        nc.vector.tensor_add(out=s[:, 0:29, :], in0=xh[:, 0:29, :], in1=xh[:, 29:58, :])
        nc.vector.tensor_add(out=s[:, 0:14, :], in0=s[:, 0:14, :], in1=s[:, 15:29, :])
        nc.vector.tensor_add(out=s[:, 0:7, :], in0=s[:, 0:7, :], in1=s[:, 7:14, :])
        nc.vector.tensor_add(out=s[:, 0:1, :], in0=s[:, 0:1, :], in1=s[:, 14:15, :])
        nc.vector.reduce_sum(
            out=pooledT[:, ih, b : b + 1],
            in_=s[:, 0:7, :],
            axis=mybir.AxisListType.XY,
        )

    def attention0():
        for ih in range(2):
            pool_half(0, ih)
        hp = s_psum.tile([HID, BL], F32, tag="sps", name="hp")
        for ih in range(2):
            nc.tensor.matmul(
                hp[:, 0:1],
                lhsT=w1_sb[:, ih, :],
                rhs=pooledT[:, ih, 0:1],
                start=(ih == 0),
                stop=(ih == 1),
            )
        attention_tail(0, 1, hp)

    def attention123():
        hp = s_psum.tile([HID, BL], F32, tag="sps", name="hp")
        for ih in range(2):
            nc.tensor.matmul(
                hp[:, :3],
                lhsT=w1_sb[:, ih, :],
                rhs=pooledT[:, ih, 1:4],
                start=(ih == 0),
                stop=(ih == 1),
            )
        attention_tail(1, 3, hp)

    # per-sample aggregation chunks: scale+add tree on DVE --
    # tensor_scalar (4x bf16) + tensor_tensor (2x bf16) beats the 1x-mode
    # fused scalar_tensor_tensor chain. Chunked by (3 kk, ot-half) so each
    # conv only waits for its own chunks and pipelines with them.
    aggs_all = {}

    def agg_chunk(b, g, ot, split=False):
        # split=True: produce the chunk one kq at a time so the first conv
        # matmul can start after a third of the DVE work (head latency).
        agg = aggp.tile(
            [128, 3, 2, 128], BF16, tag=f"agg{b}_{g}_{ot}", name=f"agg{b}_{g}_{ot}"
        )
        kqs = [(kq, kq + 1) for kq in range(3)] if split else [(0, 3)]
        for q0, q1 in kqs:
            nc.vector.tensor_scalar_mul(
                agg[:, q0:q1, :, :], walT[:, g, ot, 0, q0:q1, :, :], att_bc[:, b, 0:1]
            )
            for e in range(1, E):
                tmp = tmpp.tile([128, 3, 2, 128], BF16, tag="tmp", name="tmp")
                nc.vector.tensor_scalar_mul(
                    tmp[:, q0:q1, :, :],
                    walT[:, g, ot, e, q0:q1, :, :],
                    att_bc[:, b, e : e + 1],
                )
                nc.vector.tensor_add(
                    out=agg[:, q0:q1, :, :],
                    in0=agg[:, q0:q1, :, :],
                    in1=tmp[:, q0:q1, :, :],
                )
        for kq in range(3):
            aggs_all[(b, g * 3 + kq, ot)] = (agg, kq)

    # ---- DMA schedule: x0 first (attention head), then consts, then the
    # aligned weights in aggregation-chunk order, then the rest of x.
    dma_x(0, chunks=X0_CHUNKS)
    for ih in range(2):
        nc.sync.dma_start(out=w1_sb[:, ih, :], in_=w1_d[ih * 128 : (ih + 1) * 128, :])
    nc.sync.dma_start(out=w2_sb[:, :], in_=w2_d[:, :])
    nc.sync.dma_start(out=bias_sb[:, :], in_=bias_d[:, :])
    nc.vector.memset(ones_sb[:, :], 1.0)
    # partition starts must be 32-aligned: memset rows 64-65, relu later
    # overwrites row 64 with real h values; row 65 stays the constant 1.0.
    nc.vector.memset(h_sb[HID - 1 : HID + 1, :], 1.0)
    for ot in range(2):
        for g in range(NG):
            for e in range(E):
                nc.sync.dma_start(
                    out=walT[:, g, ot, e, :, :, :], in_=w_d[g, ot, :, e, :, :, :]
                )
    for b in range(1, BL):
        dma_x(b)

    # preload the sigmoid ACT table off the critical path (a table switch
    # costs 1.3us; done here it hides under the input DMA).
    tscr = const.tile([1, 4], F32, name="tscr")
    nc.scalar.activation(
        tscr[0:1, :], ones_sb[0:1, 0:4], mybir.ActivationFunctionType.Sigmoid
    )

    attention0()
    agg_chunk(0, 0, 0, split=True)
    for g in range(1, NG):
        agg_chunk(0, g, 0)
    for g in range(NG):
        agg_chunk(0, g, 1)
    # pooling for the other samples runs on DVE right after sample 0's
    # aggregation; their attention matmuls ride at the end of conv0 so the
    # PE never stalls on them.
    for b in range(1, BL):
        for ih in range(2):
            pool_half(b, ih)

    # ---- per-sample: aggregate weights (DVE, kk-chunked) then conv (PE) ----
    for b in range(BL):
        for ot in range(2):
            for g in range(NG):
                if (b, g * 3, ot) not in aggs_all:
                    agg_chunk(b, g, ot)

        for ot in range(2):
            # all 7 row blocks interleave across 7 PSUM banks: one weight
            # stream per (kk, ih), PE accumulates to a different bank between
            # reloads so LDWEIGHTS is amortized 7x.
            cps = {
                blk: c_psum.tile([128, RB, W], F32, tag="cps", name=f"cp{b}_{ot}_{blk}")
                for blk in range(NBLK)
            }
            for kk in range(KK):
                agg_t, kq = aggs_all[(b, kk, ot)]
                di, dj = kk // 3 - 1, kk % 3 - 1
                for ih in range(2):
                    for blk in range(NBLK):
                        r0 = blk * RB
                        nc.tensor.matmul(
                            cps[blk][:, :, :],
                            lhsT=agg_t[:, kq, ih, :],
                            rhs=x_sb[(b, ih)][
                                :,
                                r0 + di + 1 : r0 + di + 1 + RB,
                                dj + 1 : dj + 1 + W,
                            ],
                            start=(kk == 0 and ih == 0),
                            stop=(kk == KK - 1 and ih == 1),
                        )
            for blk in range(NBLK):
                r0 = blk * RB
                st = stagep.tile([128, RB, W], BF16, tag="stage", name="st")
                nc.scalar.activation(
                    st[:, :, :],
                    cps[blk][:, :, :],
                    mybir.ActivationFunctionType.Identity,
                    bias=aggb_sb[:, ot, b : b + 1],
                )
                nc.sync.dma_start(
                    out=out_d[b, ot * 128 : (ot + 1) * 128, r0 : r0 + RB, :],
                    in_=st[:, :, :],
                )
            # remaining samples' attention rides at the tail of conv0: its
            # pooling (DVE) has finished by then, so the PE matmuls slot in
            # without stalling, and agg(b+1..) overlaps the conv tails.
            if b == 0 and ot == 1:
                attention123()


def _build():
    nc = bacc.Bacc("TRN2", target_bir_lowering=False, debug=False, num_devices=N_CORES)
    with contextlib.ExitStack() as ctx:
        tc = ctx.enter_context(tile.TileContext(nc))
        _emit(nc, tc, ctx)
    nc.compile()
    return nc


def _get_nc():
    global _NC_CACHE
    if _NC_CACHE is None:
        _NC_CACHE = _build()
    return _NC_CACHE


def _run(trace=False, **inputs):
    x = np.asarray(inputs["x"], np.float32)
    weight = np.asarray(inputs["weight"], np.float32)
    bias = np.asarray(inputs["bias"], np.float32)
    align = np.asarray(inputs["align"], np.float32)
    w1 = np.asarray(inputs["attn_w1"], np.float32)
    w2 = np.asarray(inputs["attn_w2"], np.float32)
    b2 = np.asarray(inputs["attn_b2"], np.float32)

    xp = np.zeros((B, I, HP, HP), dtype=ml_dtypes.bfloat16)
    xp[:, :, 1 : 1 + H, 1 : 1 + W] = x

    # fold align into the expert weights (weight-only reparameterization),
    # then lay out transposed + chunk-ordered for the DMA schedule:
    # walT[kkg, ot, p, e, kq, ih, o] = w_al[e, ot*128+o, ih*128+p, kkg*3+kq]
    w_al = np.einsum("eno,eok->enk", align, weight.reshape(E, O, I * KK)).reshape(
        E, 2, 128, 2, 128, 3, 3
    )  # [e, ot, o, ih, p, kkg, kq]
    walT = np.ascontiguousarray(w_al.transpose(5, 1, 4, 0, 6, 3, 2)).astype(
        ml_dtypes.bfloat16
    )

    w1T = np.ascontiguousarray((w1 / float(H * W)).T)
    w2Ta = np.ascontiguousarray(
        np.concatenate([w2.T, b2.reshape(1, E)], axis=0)
    ).astype(np.float32)

    nc = _get_nc()
    in_maps = []
    for c in range(N_CORES):
        in_maps.append(
            {
                "x": xp[c * BL : (c + 1) * BL],
                "walT": walT,
                "w1T": w1T,
                "w2Ta": w2Ta,
                "bias": bias,
            }
        )
    if trace:
        _install_ntff_hook()
    res = run_bass_kernel_spmd(
        nc, in_maps, core_ids=list(range(N_CORES)), trace=trace
    )
    out = np.concatenate([res.results[c]["out"] for c in range(N_CORES)], axis=0)
    return out.astype(np.float32), res


def kernel(**inputs):
    out, _ = _run(trace=False, **inputs)
    return out


def kernel_profiled(**inputs):
    out, res = _run(trace=True, **inputs)
    return out, res


# revision 27
# speedup vs baseline: 1.2736x; 1.1988x over previous
"""AdaAggLayer Trainium2 kernel.

Data-parallel over batch: 8 NeuronCores x 4 samples each.

The align transform (w_aligned[e] = align[e] @ w[e]) is a weight-only
reparameterization, so it is folded on the host (float32, exact) and the
aligned weights are DMA'd in directly, transposed to the [i, o] layout the
conv needs. Per core:
  - attention (global avg pool -> 1x1 -> relu -> 1x1 -> sigmoid) on PE/ACT/DVE;
    sample 0's pooling is chunked so it starts while its x is still landing
  - per-sample weight aggregation sum_e att[b,e]*walT[e] on DVE (bf16,
    contiguous chunks so tensor_scalar hits 4x mode and tensor_tensor 2x)
  - per-sample 3x3 conv as 9 shifted matmuls accumulating in PSUM (bf16),
    7 row-blocks interleaved across 7 PSUM banks
  - bias epilogue fused into the PSUM->SBUF copy on ACT, output stored bf16
No collectives: inputs are sharded/replicated host-side, outputs concatenated.
"""

import contextlib
import importlib.util
import sys
import types

sys.path.insert(0, "/opt/trn_rl_repo")

import numpy as np
import ml_dtypes

import concourse.bass as bass
import concourse.mybir as mybir
import concourse.tile as tile
from concourse import bacc
from concourse.bass_utils import run_bass_kernel_spmd

N_CORES = 8
B, I, O, E, HID = 32, 256, 256, 5, 65
H = W = 56
HP = H + 2  # zero-padded spatial
BL = B // N_CORES  # samples per core
KK = 9  # 3x3 taps
NG = 3  # kk groups of 3
NBLK = 7  # row blocks of 8 output rows
RB = 8  # rows per block
BF16 = mybir.dt.bfloat16
F32 = mybir.dt.float32

# row chunks for sample 0's pooled-sum pipeline (reduce each as it lands)
X0_CHUNKS = [(0, 15), (15, 30), (30, 45), (45, 58)]

_NC_CACHE = None


def _install_ntff_hook():
    """Register the axon NTFF profiling hook (the image's antenv lacks it)."""
    if "antenv.axon_hooks" in sys.modules:
        return
    try:
        spec = importlib.util.spec_from_file_location(
            "trn_boot", "/root/.axon_site/trn_agent_boot/trn_boot.py"
        )
        tb = importlib.util.module_from_spec(spec)
        spec.loader.exec_module(tb)
        hook = tb._ntff_profile_via_ctypes("/opt/axon/libaxon_pjrt.so")
    except Exception:
        hook = None
    mod = types.ModuleType("antenv.axon_hooks")
    mod.get_axon_ntff_profile_hook = lambda: hook
    sys.modules["antenv.axon_hooks"] = mod


def _emit(nc, tc, ctx):
    x_d = nc.dram_tensor("x", [BL, I, HP, HP], BF16, kind="ExternalInput")
    # aligned+transposed weights, chunk-ordered: [kkg, ot, p=i%128, e, kq, ih, o128]
    w_d = nc.dram_tensor("walT", [NG, 2, 128, E, 3, 2, 128], BF16, kind="ExternalInput")
    # packed small constants: [p, 0:130]=w1T (2 ih), [0:66, 130:135]=w2Ta,
    # [0:5, 135:391]=bias. One DMA: the serial per-trigger cost (~650ns on
    # the sync sequencer) dominates small transfers.
    misc_d = nc.dram_tensor("misc", [128, 391], F32, kind="ExternalInput")
    out_d = nc.dram_tensor("out", [BL, O, H, W], BF16, kind="ExternalOutput")

    const = ctx.enter_context(tc.tile_pool(name="const", bufs=1))
    xpool = ctx.enter_context(tc.tile_pool(name="x", bufs=1))
    aggp = ctx.enter_context(tc.tile_pool(name="agg", bufs=1))
    tmpp = ctx.enter_context(tc.tile_pool(name="tmp", bufs=2))
    stagep = ctx.enter_context(tc.tile_pool(name="stage", bufs=4))
    s_psum = ctx.enter_context(tc.tile_pool(name="sps", bufs=1, space="PSUM"))
    c_psum = ctx.enter_context(tc.tile_pool(name="cps", bufs=7, space="PSUM"))

    # ---- SBUF constants ----
    # walT[p, kkg, ot, e, kq, ih, o]: each (kkg, ot, e) run is contiguous so
    # the aggregation DVE ops stream stride-1 (4x/2x perf modes).
    walT = const.tile([128, NG, 2, E, 3, 2, 128], BF16)
    pscr = const.tile([128, 2, 29, HP], BF16)  # pooling tree-reduce scratch
    misc_sb = const.tile([128, 391], F32)
    w1_v = lambda ih: misc_sb[:, ih * HID : (ih + 1) * HID]
    w2_sb = misc_sb[: HID + 1, 130:135]
    bias_sb = misc_sb[:E, 135:391]
    ones_sb = const.tile([1, 128], F32)
    pooledT = const.tile([128, 2, BL], F32)  # part = i % 128
    h_sb = const.tile([HID + 1, BL], F32)  # row HID is constant 1.0
    att_sb = const.tile([E, BL], F32)
    att_row = const.tile([1, BL * E], F32)
    att_bc = const.tile([128, BL, E], F32)
    aggb_sb = const.tile([128, 2, BL], F32)  # part = o % 128

    x_sb = {}

    def dma_x(b, chunks=None):
        for ih in range(2):
            t = xpool.tile([128, HP, HP], BF16, tag=f"x{b}_{ih}", name=f"x{b}_{ih}")
            if chunks is None:
                nc.sync.dma_start(out=t[:, :, :], in_=x_d[b, ih * 128 : (ih + 1) * 128, :, :])
            else:
                for r0, r1 in chunks:
                    nc.sync.dma_start(
                        out=t[:, r0:r1, :],
                        in_=x_d[b, ih * 128 : (ih + 1) * 128, r0:r1, :],
                    )
            x_sb[(b, ih)] = t

    def attention_tail(b0, nb, hp):
        # shared epilogue once hp[:, :nb] holds the pre-relu hidden acts
        nc.scalar.activation(
            h_sb[:HID, b0 : b0 + nb], hp[:, :nb], mybir.ActivationFunctionType.Relu
        )
        ap = s_psum.tile([E, BL], F32, tag="sps", name="ap")
        nc.tensor.matmul(ap[:, :nb], lhsT=w2_sb[:, :], rhs=h_sb[:, b0 : b0 + nb])
        nc.scalar.activation(
            att_sb[:, b0 : b0 + nb], ap[:, :nb], mybir.ActivationFunctionType.Sigmoid
        )
        # att row per sample on partition 0 (M=1 matmul), then one broadcast
        # matmul to all 128 partitions -- no SBUF-to-SBUF DMA involved.
        rp = s_psum.tile([1, BL * E], F32, tag="sps", name="rp")
        for j in range(nb):
            b = b0 + j
            nc.tensor.matmul(
                rp[0:1, j * E : (j + 1) * E],
                lhsT=h_sb[:, b : b + 1],
                rhs=w2_sb[:, :],
            )
        nc.scalar.activation(
            att_row[0:1, b0 * E : (b0 + nb) * E],
            rp[0:1, : nb * E],
            mybir.ActivationFunctionType.Sigmoid,
        )
        bp = s_psum.tile([128, BL * E], F32, tag="sps", name="bp")
        nc.tensor.matmul(
            bp[:, : nb * E],
            lhsT=ones_sb[0:1, :],
            rhs=att_row[0:1, b0 * E : (b0 + nb) * E],
        )
        nc.vector.tensor_copy(out=att_bc[:, b0 : b0 + nb, :], in_=bp[:, : nb * E])
        # aggregated bias agg_b[o, b] = sum_e att[e,b] * bias[e, o]
        for ot in range(2):
            gp = s_psum.tile([128, BL], F32, tag="sps", name="gp")
            nc.tensor.matmul(
                gp[:, :nb],
                lhsT=bias_sb[:, ot * 128 : (ot + 1) * 128],
                rhs=att_sb[:, b0 : b0 + nb],
            )
            nc.vector.tensor_copy(out=aggb_sb[:, ot, b0 : b0 + nb], in_=gp[:, :nb])

    def pool_half(b, ih):
        # pooled sum as a pairwise row-fold tree: the adds run in 2x bf16 DVE
        # mode, beating the 1x tensor_reduce (~2.5us vs 3.6us per half).
        s = pscr[:, ih]
        xh = x_sb[(b, ih)]
        nc.vector.tensor_add(out=s[:, 0:29, :], in0=xh[:, 0:29, :], in1=xh[:, 29:58, :])
        nc.vector.tensor_add(out=s[:, 0:14, :], in0=s[:, 0:14, :], in1=s[:, 15:29, :])
        nc.vector.tensor_add(out=s[:, 0:7, :], in0=s[:, 0:7, :], in1=s[:, 7:14, :])
        nc.vector.tensor_add(out=s[:, 0:1, :], in0=s[:, 0:1, :], in1=s[:, 14:15, :])
        nc.vector.reduce_sum(
            out=pooledT[:, ih, b : b + 1],
            in_=s[:, 0:7, :],
            axis=mybir.AxisListType.XY,
        )

    def attention0():
        for ih in range(2):
            pool_half(0, ih)
        hp = s_psum.tile([HID, BL], F32, tag="sps", name="hp")
        for ih in range(2):
            nc.tensor.matmul(
                hp[:, 0:1],
                lhsT=w1_v(ih),
                rhs=pooledT[:, ih, 0:1],
                start=(ih == 0),
                stop=(ih == 1),
            )
        attention_tail(0, 1, hp)

    def attention_n(b0, nb):
        hp = s_psum.tile([HID, BL], F32, tag="sps", name="hp")
        for ih in range(2):
            nc.tensor.matmul(
                hp[:, :nb],
                lhsT=w1_v(ih),
                rhs=pooledT[:, ih, b0 : b0 + nb],
                start=(ih == 0),
                stop=(ih == 1),
            )
        attention_tail(b0, nb, hp)

    # per-sample aggregation chunks: scale+add tree on DVE --
    # tensor_scalar (4x bf16) + tensor_tensor (2x bf16) beats the 1x-mode
    # fused scalar_tensor_tensor chain. Chunked by (3 kk, ot-half) so each
    # conv only waits for its own chunks and pipelines with them.
    aggs_all = {}

    def agg_chunk(b, g, ot, split=False):
        # split=True: produce the chunk one kq at a time so the first conv
        # matmul can start after a third of the DVE work (head latency).
        agg = aggp.tile(
            [128, 3, 2, 128], BF16, tag=f"agg{b}_{g}_{ot}", name=f"agg{b}_{g}_{ot}"
        )
        kqs = [(kq, kq + 1) for kq in range(3)] if split else [(0, 3)]
        for q0, q1 in kqs:
            nc.vector.tensor_scalar_mul(
                agg[:, q0:q1, :, :], walT[:, g, ot, 0, q0:q1, :, :], att_bc[:, b, 0:1]
            )
            for e in range(1, E):
                tmp = tmpp.tile([128, 3, 2, 128], BF16, tag="tmp", name="tmp")
                nc.vector.tensor_scalar_mul(
                    tmp[:, q0:q1, :, :],
                    walT[:, g, ot, e, q0:q1, :, :],
                    att_bc[:, b, e : e + 1],
                )
                nc.vector.tensor_add(
                    out=agg[:, q0:q1, :, :],
                    in0=agg[:, q0:q1, :, :],
                    in1=tmp[:, q0:q1, :, :],
                )
        for kq in range(3):
            aggs_all[(b, g * 3 + kq, ot)] = (agg, kq)

    # ---- DMA schedule: x0 first (attention head), then consts, then the
    # aligned weights in aggregation-chunk order, then the rest of x.
    # Few, large DMAs: each dma_start costs ~650ns of serial trigger issue
    # on the sync sequencer, so the head is trigger-limited, not BW-limited.
    dma_x(0)
    nc.sync.dma_start(out=misc_sb[:, :], in_=misc_d[:, :])
    nc.vector.memset(ones_sb[:, :], 1.0)
    # partition starts must be 32-aligned: memset rows 64-65, relu later
    # overwrites row 64 with real h values; row 65 stays the constant 1.0.
    nc.vector.memset(h_sb[HID - 1 : HID + 1, :], 1.0)
    for ot in range(2):
        for g in range(NG):
            nc.sync.dma_start(out=walT[:, g, ot], in_=w_d[g, ot])
    for b in range(1, BL):
        dma_x(b)

    # preload the sigmoid ACT table off the critical path (a table switch
    # costs 1.3us; done here it hides under the input DMA).
    tscr = const.tile([1, 4], F32, name="tscr")
    nc.scalar.activation(
        tscr[0:1, :], ones_sb[0:1, 0:4], mybir.ActivationFunctionType.Sigmoid
    )

    attention0()
    agg_chunk(0, 0, 0, split=True)
    for g in range(1, NG):
        agg_chunk(0, g, 0)
    for g in range(NG):
        agg_chunk(0, g, 1)
    # pooling for the other samples runs on DVE right after sample 0's
    # aggregation; their attention matmuls are hooked into the conv stream
    # (b1 after conv0-ot0, b2/b3 after conv1-ot0) so agg(b+1) is always
    # ready before conv(b+1) and the PE never stalls.
    for b in range(1, BL):
        for ih in range(2):
            pool_half(b, ih)

    # ---- per-sample: aggregate weights (DVE, kk-chunked) then conv (PE) ----
    for b in range(BL):
        for ot in range(2):
            for g in range(NG):
                if (b, g * 3, ot) not in aggs_all:
                    agg_chunk(b, g, ot)

        for ot in range(2):
            # all 7 row blocks interleave across 7 PSUM banks: one weight
            # stream per (kk, ih), PE accumulates to a different bank between
            # reloads so LDWEIGHTS is amortized 7x.
            cps = {
                blk: c_psum.tile([128, RB, W], F32, tag="cps", name=f"cp{b}_{ot}_{blk}")
                for blk in range(NBLK)
            }
            for kk in range(KK):
                agg_t, kq = aggs_all[(b, kk, ot)]
                di, dj = kk // 3 - 1, kk % 3 - 1
                for ih in range(2):
                    for blk in range(NBLK):
                        r0 = blk * RB
                        nc.tensor.matmul(
                            cps[blk][:, :, :],
                            lhsT=agg_t[:, kq, ih, :],
                            rhs=x_sb[(b, ih)][
                                :,
                                r0 + di + 1 : r0 + di + 1 + RB,
                                dj + 1 : dj + 1 + W,
                            ],
                            start=(kk == 0 and ih == 0),
                            stop=(kk == KK - 1 and ih == 1),
                        )
            for blk in range(NBLK):
                r0 = blk * RB
                st = stagep.tile([128, RB, W], BF16, tag="stage", name="st")
                nc.scalar.activation(
                    st[:, :, :],
                    cps[blk][:, :, :],
                    mybir.ActivationFunctionType.Identity,
                    bias=aggb_sb[:, ot, b : b + 1],
                )
                # trigger the store from the scalar engine: sync's serial
                # ~650ns-per-trigger issue stays reserved for input DMAs.
                nc.scalar.dma_start(
                    out=out_d[b, ot * 128 : (ot + 1) * 128, r0 : r0 + RB, :],
                    in_=st[:, :, :],
                )
            if b == 0 and ot == 0:
                attention_n(1, 1)
            if b == 1 and ot == 0:
                attention_n(2, 2)


def _build():
    nc = bacc.Bacc("TRN2", target_bir_lowering=False, debug=False, num_devices=N_CORES)
    with contextlib.ExitStack() as ctx:
        tc = ctx.enter_context(tile.TileContext(nc))
        _emit(nc, tc, ctx)
    nc.compile()
    return nc


def _get_nc():
    global _NC_CACHE
    if _NC_CACHE is None:
        _NC_CACHE = _build()
    return _NC_CACHE


def _run(trace=False, **inputs):
    x = np.asarray(inputs["x"], np.float32)
    weight = np.asarray(inputs["weight"], np.float32)
    bias = np.asarray(inputs["bias"], np.float32)
    align = np.asarray(inputs["align"], np.float32)
    w1 = np.asarray(inputs["attn_w1"], np.float32)
    w2 = np.asarray(inputs["attn_w2"], np.float32)
    b2 = np.asarray(inputs["attn_b2"], np.float32)

    xp = np.zeros((B, I, HP, HP), dtype=ml_dtypes.bfloat16)
    xp[:, :, 1 : 1 + H, 1 : 1 + W] = x

    # fold align into the expert weights (weight-only reparameterization),
    # then lay out transposed + chunk-ordered for the DMA schedule:
    # walT[kkg, ot, p, e, kq, ih, o] = w_al[e, ot*128+o, ih*128+p, kkg*3+kq]
    w_al = np.einsum("eno,eok->enk", align, weight.reshape(E, O, I * KK)).reshape(
        E, 2, 128, 2, 128, 3, 3
    )  # [e, ot, o, ih, p, kkg, kq]
    walT = np.ascontiguousarray(w_al.transpose(5, 1, 4, 0, 6, 3, 2)).astype(
        ml_dtypes.bfloat16
    )

    w1T = (w1 / float(H * W)).T.reshape(2, 128, HID)  # [ih, p, HID]
    w2Ta = np.concatenate([w2.T, b2.reshape(1, E)], axis=0)  # [66, E]
    misc = np.zeros((128, 391), np.float32)
    misc[:, 0:HID] = w1T[0]
    misc[:, HID : 2 * HID] = w1T[1]
    misc[: HID + 1, 130:135] = w2Ta
    misc[:E, 135:391] = bias

    nc = _get_nc()
    in_maps = []
    for c in range(N_CORES):
        in_maps.append(
            {
                "x": xp[c * BL : (c + 1) * BL],
                "walT": walT,
                "misc": misc,
            }
        )
    if trace:
        _install_ntff_hook()
    res = run_bass_kernel_spmd(
        nc, in_maps, core_ids=list(range(N_CORES)), trace=trace
    )
    out = np.concatenate([res.results[c]["out"] for c in range(N_CORES)], axis=0)
    return out.astype(np.float32), res


def kernel(**inputs):
    out, _ = _run(trace=False, **inputs)
    return out


def kernel_profiled(**inputs):
    out, res = _run(trace=True, **inputs)
    return out, res


# revision 29
# speedup vs baseline: 1.2948x; 1.0167x over previous
"""AdaAggLayer Trainium2 kernel — 1D Winograd F(2,3) along W.

Data-parallel over batch: 8 NeuronCores x 4 samples each.

The 3x3 conv is decomposed as Winograd F(2,3) along the width axis only:
per (kh row, output-column pair) the 3 kw taps become 4 Winograd taps, so
the PE does 12 tap-matmuls per output instead of 18 shifted matmuls — a
1.5x cut in TensorE cycles (the roofline engine). The tap transform of the
weights rides the host-side align fold (both are weight-only, exact f32);
aggregation then happens directly in tap space on DVE. The input taps
  t0 = xe[j]-xe[j+1], t1 = xo[j]+xe[j+1], t2 = xe[j+1]-xo[j],
  t3 = xo[j]-xo[j+1]
are built from host-split even/odd column planes (pure layout) as
contiguous tensor_tensor adds on DVE (never GpSimd: concurrent Pool
tensor ops degrade DVE ~6x via SBUF contention). The inverse transform
  y_even = e0+e1+e2, y_odd = e1-e2-e3
runs on DVE in bf16 straight off the ACT evacuations (bias rides e1's
evac since its coefficient is +1 in both outputs). Output is stored as
[h, parity, w'] and interleaved on the host (pure layout).
"""

import contextlib
import importlib.util
import sys
import types

sys.path.insert(0, "/opt/trn_rl_repo")

import numpy as np
import ml_dtypes

import concourse.bass as bass
import concourse.mybir as mybir
import concourse.tile as tile
from concourse import bacc
from concourse.bass_utils import run_bass_kernel_spmd

N_CORES = 8
B, I, O, E, HID = 32, 256, 256, 5, 65
H = W = 56
HP = H + 2  # zero-padded spatial rows
WE = 29  # even/odd column plane width (padded 58 cols split)
WT = 28  # winograd output-pair columns
KH = 3
TAP = 4
NBLK = 4  # row blocks of 14 output rows
RB = 14
BF16 = mybir.dt.bfloat16
F32 = mybir.dt.float32

_NC_CACHE = None


def _install_ntff_hook():
    """Register the axon NTFF profiling hook (the image's antenv lacks it)."""
    if "antenv.axon_hooks" in sys.modules:
        return
    try:
        spec = importlib.util.spec_from_file_location(
            "trn_boot", "/root/.axon_site/trn_agent_boot/trn_boot.py"
        )
        tb = importlib.util.module_from_spec(spec)
        spec.loader.exec_module(tb)
        hook = tb._ntff_profile_via_ctypes("/opt/axon/libaxon_pjrt.so")
    except Exception:
        hook = None
    mod = types.ModuleType("antenv.axon_hooks")
    mod.get_axon_ntff_profile_hook = lambda: hook
    sys.modules["antenv.axon_hooks"] = mod


def _emit(nc, tc, ctx):
    xe_d = nc.dram_tensor("xe", [4, I, HP, WE], BF16, kind="ExternalInput")
    xo_d = nc.dram_tensor("xo", [4, I, HP, WE], BF16, kind="ExternalInput")
    # tap-transformed aligned weights, chunk-ordered:
    # [kh, ot, p=i%128, e, tap, ih, o128]
    w_d = nc.dram_tensor("wt", [KH, 2, 128, E, TAP, 2, 128], BF16, kind="ExternalInput")
    # packed small constants: [p, 0:130]=w1T (2 ih), [0:66, 130:135]=w2Ta,
    # [0:5, 135:391]=bias. One DMA: each dma_start costs ~650ns of serial
    # trigger issue on the sync sequencer.
    misc_d = nc.dram_tensor("misc", [128, 391], F32, kind="ExternalInput")
    out_d = nc.dram_tensor("out", [4, O, H, 2, WT], BF16, kind="ExternalOutput")

    const = ctx.enter_context(tc.tile_pool(name="const", bufs=1))
    xpl = ctx.enter_context(tc.tile_pool(name="xpl", bufs=1))
    xtp = ctx.enter_context(tc.tile_pool(name="xt", bufs=1))
    aggp = ctx.enter_context(tc.tile_pool(name="agg", bufs=1))
    tmpp = ctx.enter_context(tc.tile_pool(name="tmp", bufs=2))
    evp = ctx.enter_context(tc.tile_pool(name="ev", bufs=1))
    ytp = ctx.enter_context(tc.tile_pool(name="yt", bufs=3))
    stagep = ctx.enter_context(tc.tile_pool(name="stage", bufs=4))
    s_psum = ctx.enter_context(tc.tile_pool(name="sps", bufs=1, space="PSUM"))
    t_psum = ctx.enter_context(tc.tile_pool(name="tps", bufs=7, space="PSUM"))

    # ---- SBUF constants ----
    walTt = const.tile([128, KH, 2, E, TAP, 2, 128], BF16)
    pscr = const.tile([128, 2, 29, WE], BF16)  # pooling tree scratch
    misc_sb = const.tile([128, 391], F32)
    w1_v = lambda ih: misc_sb[:, ih * HID : (ih + 1) * HID]
    w2_sb = misc_sb[: HID + 1, 130:135]
    bias_sb = misc_sb[:E, 135:391]
    ones_sb = const.tile([1, 128], F32)
    pooledT = const.tile([128, 2, 4], F32)
    h_sb = const.tile([HID + 1, 4], F32)  # row HID is constant 1.0
    att_sb = const.tile([E, 4], F32)
    att_row = const.tile([1, 4 * E], F32)
    att_bc = const.tile([128, 4, E], F32)
    aggb_sb = const.tile([128, 2, 4], F32)

    xe_sb = {}
    xt_sb = {}

    def dma_xplanes(b):
        for ih in range(2):
            te = xpl.tile([128, HP, WE], BF16, tag=f"xe_{ih}", bufs=2, name=f"xe{b}_{ih}")
            to = xpl.tile([128, HP, WE], BF16, tag=f"xo_{ih}", bufs=2, name=f"xo{b}_{ih}")
            nc.sync.dma_start(out=te[:, :, :], in_=xe_d[b, ih * 128 : (ih + 1) * 128])
            nc.sync.dma_start(out=to[:, :, :], in_=xo_d[b, ih * 128 : (ih + 1) * 128])
            xe_sb[(b, ih)] = (te, to)

    def build_xt_taps(b, ih, taps):
        # winograd input taps as contiguous tensor_tensor ops (DVE 2x).
        # NOTE: never place these on gpsimd — concurrent Pool tensor ops
        # degrade DVE throughput ~6x (SBUF contention).
        if (b, ih) in xt_sb:
            t = xt_sb[(b, ih)]
        else:
            t = xtp.tile(
                [128, TAP, HP, WT], BF16, tag=f"xt_{ih}", bufs=2, name=f"xt{b}_{ih}"
            )
            xt_sb[(b, ih)] = t
        xe, xo = xe_sb[(b, ih)]
        for tap in taps:
            if tap == 0:
                nc.vector.tensor_sub(out=t[:, 0], in0=xe[:, :, 0:28], in1=xe[:, :, 1:29])
            elif tap == 1:
                nc.vector.tensor_add(out=t[:, 1], in0=xo[:, :, 0:28], in1=xe[:, :, 1:29])
            elif tap == 2:
                nc.vector.tensor_sub(out=t[:, 2], in0=xe[:, :, 1:29], in1=xo[:, :, 0:28])
            else:
                nc.vector.tensor_sub(out=t[:, 3], in0=xo[:, :, 0:28], in1=xo[:, :, 1:29])

    def _pool_tree(b, ih, ncols):
        s = pscr[:, ih]
        nc.vector.tensor_add(
            out=s[:, 0:14, 0:ncols], in0=s[:, 0:14, 0:ncols], in1=s[:, 15:29, 0:ncols]
        )
        nc.vector.tensor_add(
            out=s[:, 0:7, 0:ncols], in0=s[:, 0:7, 0:ncols], in1=s[:, 7:14, 0:ncols]
        )
        nc.vector.tensor_add(
            out=s[:, 0:1, 0:ncols], in0=s[:, 0:1, 0:ncols], in1=s[:, 14:15, 0:ncols]
        )
        nc.vector.reduce_sum(
            out=pooledT[:, ih, b : b + 1],
            in_=s[:, 0:7, 0:ncols],
            axis=mybir.AxisListType.XY,
        )

    def pool_half_raw(b, ih):
        # pooling for samples whose x_t is not built yet (avoids blocking the
        # DVE queue on the x_t tile ring): fold xe+xo then the row tree.
        s = pscr[:, ih]
        xe, xo = xe_sb[(b, ih)]
        nc.vector.tensor_add(out=s[:, 0:29, :], in0=xe[:, 0:29, :], in1=xe[:, 29:58, :])
        nc.vector.tensor_add(out=s[:, 0:29, :], in0=s[:, 0:29, :], in1=xo[:, 0:29, :])
        nc.vector.tensor_add(out=s[:, 0:29, :], in0=s[:, 0:29, :], in1=xo[:, 29:58, :])
        _pool_tree(b, ih, WE)

    def pool_half(b, ih):
        # pooled sum from winograd tap 1: sum_j (xo[j]+xe[j+1]) telescopes to
        # the full (zero-padded) row sum, so the tap plane doubles as the
        # pooling input. Pairwise row-fold tree in 2x bf16.
        s = pscr[:, ih]
        t1 = xt_sb[(b, ih)][:, 1]
        nc.vector.tensor_add(
            out=s[:, 0:29, 0:WT], in0=t1[:, 0:29, :], in1=t1[:, 29:58, :]
        )
        _pool_tree(b, ih, WT)

    def attention_tail(b0, nb, hp):
        nc.scalar.activation(
            h_sb[:HID, b0 : b0 + nb], hp[:, :nb], mybir.ActivationFunctionType.Relu
        )
        ap = s_psum.tile([E, 4], F32, tag="sps", name="ap")
        nc.tensor.matmul(ap[:, :nb], lhsT=w2_sb[:, :], rhs=h_sb[:, b0 : b0 + nb])
        nc.scalar.activation(
            att_sb[:, b0 : b0 + nb], ap[:, :nb], mybir.ActivationFunctionType.Sigmoid
        )
        rp = s_psum.tile([1, 4 * E], F32, tag="sps", name="rp")
        for j in range(nb):
            nc.tensor.matmul(
                rp[0:1, j * E : (j + 1) * E],
                lhsT=h_sb[:, b0 + j : b0 + j + 1],
                rhs=w2_sb[:, :],
            )
        nc.scalar.activation(
            att_row[0:1, b0 * E : (b0 + nb) * E],
            rp[0:1, : nb * E],
            mybir.ActivationFunctionType.Sigmoid,
        )
        bp = s_psum.tile([128, 4 * E], F32, tag="sps", name="bp")
        nc.tensor.matmul(
            bp[:, : nb * E],
            lhsT=ones_sb[0:1, :],
            rhs=att_row[0:1, b0 * E : (b0 + nb) * E],
        )
        nc.vector.tensor_copy(out=att_bc[:, b0 : b0 + nb, :], in_=bp[:, : nb * E])
        for ot in range(2):
            gp = s_psum.tile([128, 4], F32, tag="sps", name="gp")
            nc.tensor.matmul(
                gp[:, :nb],
                lhsT=bias_sb[:, ot * 128 : (ot + 1) * 128],
                rhs=att_sb[:, b0 : b0 + nb],
            )
            nc.vector.tensor_copy(out=aggb_sb[:, ot, b0 : b0 + nb], in_=gp[:, :nb])

    def attention_n(b0, nb):
        hp = s_psum.tile([HID, 4], F32, tag="sps", name="hp")
        for ih in range(2):
            nc.tensor.matmul(
                hp[:, :nb],
                lhsT=w1_v(ih),
                rhs=pooledT[:, ih, b0 : b0 + nb],
                start=(ih == 0),
                stop=(ih == 1),
            )
        attention_tail(b0, nb, hp)

    # per-sample tap-space aggregation on DVE: tensor_scalar 4x muls + 2x adds
    aggs_all = {}

    def agg_chunk(b, kh, ot, split=False):
        # experts 1,2 scaled on ACT (activation Identity with per-partition
        # scale) to shed DVE work; DVE does the other muls (4x) + adds (2x).
        agg = aggp.tile(
            [128, TAP, 2, 128], BF16, tag=f"agg_{kh}_{ot}", bufs=2, name=f"agg{b}_{kh}_{ot}"
        )
        parts = [(0, 2), (2, 4)] if split else [(0, 4)]
        for q0, q1 in parts:
            acts = {}
            for e in (1, 2):
                ta = tmpp.tile([128, TAP, 2, 128], BF16, tag=f"tmpa{e}", name="ta")
                nc.scalar.activation(
                    ta[:, q0:q1],
                    walTt[:, kh, ot, e, q0:q1],
                    mybir.ActivationFunctionType.Identity,
                    scale=att_bc[:, b, e : e + 1],
                )
                acts[e] = ta
            nc.vector.tensor_scalar_mul(
                agg[:, q0:q1], walTt[:, kh, ot, 0, q0:q1], att_bc[:, b, 0:1]
            )
            for e in (3, 4):
                tmp = tmpp.tile(
                    [128, TAP, 2, 128], BF16, tag=f"tmp{e}", bufs=1, name="tmp"
                )
                nc.vector.tensor_scalar_mul(
                    tmp[:, q0:q1], walTt[:, kh, ot, e, q0:q1], att_bc[:, b, e : e + 1]
                )
                acts[e] = tmp
            for e in (1, 2, 3, 4):
                nc.vector.tensor_add(
                    out=agg[:, q0:q1], in0=agg[:, q0:q1], in1=acts[e][:, q0:q1]
                )
        aggs_all[(b, kh, ot)] = agg

    # ---- DMA schedule ----
    dma_xplanes(0)
    nc.sync.dma_start(out=misc_sb[:, :], in_=misc_d[:, :])
    nc.vector.memset(ones_sb[:, :], 1.0)
    nc.vector.memset(h_sb[HID - 1 : HID + 1, :], 1.0)
    for ot in range(2):
        for kh in range(KH):
            nc.sync.dma_start(out=walTt[:, kh, ot], in_=w_d[kh, ot])
    dma_xplanes(1)

    # preload the sigmoid ACT table off the critical path
    tscr = const.tile([1, 4], F32, name="tscr")
    nc.scalar.activation(
        tscr[0:1, :], ones_sb[0:1, 0:4], mybir.ActivationFunctionType.Sigmoid
    )

    for ih in range(2):
        build_xt_taps(0, ih, [1])
    for ih in range(2):
        pool_half(0, ih)
    attention_n(0, 1)
    for ih in range(2):
        build_xt_taps(0, ih, [0, 2, 3])
    agg_chunk(0, 0, 0, split=True)
    agg_chunk(0, 1, 0)
    agg_chunk(0, 2, 0)
    for ih in range(2):
        build_xt_taps(1, ih, [1])
    for ih in range(2):
        pool_half(1, ih)
    for kh in range(KH):
        agg_chunk(0, kh, 1)

    # ---- per-sample winograd conv ----
    for b in range(4):
        if b >= 1:
            for ot in range(2):
                for kh in range(KH):
                    agg_chunk(b, kh, ot)
        for ot in range(2):
            for pair in range(2):
                ev = {
                    tap: evp.tile(
                        [128, 2, RB, WT], BF16, tag=f"e{tap}", bufs=2, name=f"e{tap}"
                    )
                    for tap in range(TAP)
                }
                for sub in range(2):
                    blk = pair * 2 + sub
                    r0 = blk * RB
                    tp = {
                        tap: t_psum.tile([128, RB, WT], F32, tag="tap", name=f"tp{tap}")
                        for tap in range(TAP)
                    }
                    for kh in range(KH):
                        agg = aggs_all[(b, kh, ot)]
                        for tap in range(TAP):
                            for ih in range(2):
                                nc.tensor.matmul(
                                    tp[tap][:, :, :],
                                    lhsT=agg[:, tap, ih, :],
                                    rhs=xt_sb[(b, ih)][
                                        :, tap, r0 + kh : r0 + kh + RB, :
                                    ],
                                    start=(kh == 0 and ih == 0),
                                    stop=(kh == KH - 1 and ih == 1),
                                )
                    # evac taps to bf16; bias rides e1 (coefficient +1 in
                    # both winograd outputs)
                    for tap in range(TAP):
                        if tap == 1:
                            nc.scalar.activation(
                                ev[tap][:, sub],
                                tp[tap][:, :, :],
                                mybir.ActivationFunctionType.Identity,
                                bias=aggb_sb[:, ot, b : b + 1],
                            )
                        else:
                            nc.scalar.activation(
                                ev[tap][:, sub],
                                tp[tap][:, :, :],
                                mybir.ActivationFunctionType.Identity,
                            )
                # inverse transform on DVE (bf16 2x), both blocks at once,
                # into the parity-split stage
                st = stagep.tile([128, 2, RB, 2, WT], BF16, tag="stage", bufs=3, name="st")
                y0t = ytp.tile([128, 2, RB, WT], BF16, tag="yt", name="y0t")
                nc.vector.tensor_add(out=y0t, in0=ev[0][:, :, :, :], in1=ev[1][:, :, :, :])
                nc.vector.tensor_add(out=st[:, :, :, 0, :], in0=y0t[:, :, :, :], in1=ev[2][:, :, :, :])
                y1t = ytp.tile([128, 2, RB, WT], BF16, tag="yt", name="y1t")
                nc.vector.tensor_sub(out=y1t, in0=ev[1][:, :, :, :], in1=ev[2][:, :, :, :])
                nc.vector.tensor_sub(out=st[:, :, :, 1, :], in0=y1t[:, :, :, :], in1=ev[3][:, :, :, :])
                nc.sync.dma_start(
                    out=out_d[b, ot * 128 : (ot + 1) * 128, pair * 2 * RB : (pair + 1) * 2 * RB, :, :],
                    in_=st[:, :, :, :, :],
                )
            # pipeline hooks: next sample's attention + remaining input taps
            # after ot0; the sample after that gets its tap-1 plane + pooling
            # after ot1 (its x lands mid-conv).
            if ot == 0 and b < 3:
                attention_n(b + 1, 1)
                for ih in range(2):
                    build_xt_taps(b + 1, ih, [0, 2, 3])
                if b + 2 < 4:
                    dma_xplanes(b + 2)
            if ot == 1 and b < 2:
                for ih in range(2):
                    build_xt_taps(b + 2, ih, [1])
                for ih in range(2):
                    pool_half(b + 2, ih)


def _build():
    nc = bacc.Bacc("TRN2", target_bir_lowering=False, debug=False, num_devices=N_CORES)
    with contextlib.ExitStack() as ctx:
        tc = ctx.enter_context(tile.TileContext(nc))
        _emit(nc, tc, ctx)
    nc.compile()
    return nc


def _get_nc():
    global _NC_CACHE
    if _NC_CACHE is None:
        _NC_CACHE = _build()
    return _NC_CACHE


def _run(trace=False, **inputs):
    BL = 4
    x = np.asarray(inputs["x"], np.float32)
    weight = np.asarray(inputs["weight"], np.float32)
    bias = np.asarray(inputs["bias"], np.float32)
    align = np.asarray(inputs["align"], np.float32)
    w1 = np.asarray(inputs["attn_w1"], np.float32)
    w2 = np.asarray(inputs["attn_w2"], np.float32)
    b2 = np.asarray(inputs["attn_b2"], np.float32)

    xp = np.zeros((B, I, HP, HP), dtype=ml_dtypes.bfloat16)
    xp[:, :, 1 : 1 + H, 1 : 1 + W] = x
    xe = np.ascontiguousarray(xp[:, :, :, 0::2])
    xo = np.ascontiguousarray(xp[:, :, :, 1::2])

    # host: fold align (weight-only reparam) + winograd kw-tap transform,
    # then lay out chunk-ordered:
    # wt[kh, ot, p, e, tap, ih, o] = wt_al[e, ot*128+o, ih*128+p, kh, tap]
    w_al = np.einsum("eno,eok->enk", align, weight.reshape(E, O, I * 9)).reshape(
        E, O, I, 3, 3
    )
    T = np.array(
        [[1, 0, 0], [0.5, 0.5, 0.5], [0.5, -0.5, 0.5], [0, 0, 1]], np.float32
    )
    wt = np.einsum("tk,enihk->eniht", T, w_al)  # [E, O, I, KH, TAP]
    wt = wt.reshape(E, 2, 128, 2, 128, KH, TAP)  # [e, ot, o, ih, p, kh, tap]
    wt = np.ascontiguousarray(wt.transpose(5, 1, 4, 0, 6, 3, 2)).astype(
        ml_dtypes.bfloat16
    )

    w1T = (w1 / float(H * W)).T.reshape(2, 128, HID)
    w2Ta = np.concatenate([w2.T, b2.reshape(1, E)], axis=0)
    misc = np.zeros((128, 391), np.float32)
    misc[:, 0:HID] = w1T[0]
    misc[:, HID : 2 * HID] = w1T[1]
    misc[: HID + 1, 130:135] = w2Ta
    misc[:E, 135:391] = bias

    nc = _get_nc()
    in_maps = []
    for c in range(N_CORES):
        in_maps.append(
            {
                "xe": xe[c * BL : (c + 1) * BL],
                "xo": xo[c * BL : (c + 1) * BL],
                "wt": wt,
                "misc": misc,
            }
        )
    if trace:
        _install_ntff_hook()
    res = run_bass_kernel_spmd(
        nc, in_maps, core_ids=list(range(N_CORES)), trace=trace
    )
    out = np.concatenate([res.results[c]["out"] for c in range(N_CORES)], axis=0)
    # interleave the parity planes: [B,O,H,2,28] -> [B,O,H,56]
    out = out.transpose(0, 1, 2, 4, 3).reshape(B, O, H, W)
    return out.astype(np.float32), res


def kernel(**inputs):
    out, _ = _run(trace=False, **inputs)
    return out


def kernel_profiled(**inputs):
    out, res = _run(trace=True, **inputs)
    return out, res
